# revision 5
# baseline (speedup 1.0000x reference)
"""DenseNibblePPR diffusion kernel for 8 Trainium2 NeuronCores.

Math: out = ppr[idx] @ (X @ W + b),  shapes:
  X [16384, 128] f32, ppr [16384, 16384] f32, W [128, 64] f32,
  b [64] f32, idx [4096] i64  ->  out [4096, 64] f32.

Sharding strategy (batch / seed-node parallel):
  The 4096 seed nodes are split across the 8 cores (512 each). Each
  core receives its 512 gathered PPR rows, pre-transposed to
  [16384, 512] so the contraction dim (nodes) lands on SBUF
  partitions, plus a transposed copy of X ([128, 16384]) so the
  encoder matmul needs no on-chip transpose. Each core computes the
  full encoder enc = X @ W + b on-device ([16384, 64]) and then the
  local diffusion GEMM outT = enc^T-contracted rowsT -> [64, 512] in
  a single PSUM accumulation chain over 128 k-chunks. The host
  concatenates the per-core [512, 64] results. No collectives needed.
"""

import numpy as np

N = 16384
D_IN = 128
D_H = 64
B = 4096
N_CORES = 8
B_LOC = B // N_CORES  # 512
KC = N // 128  # 128 contraction chunks of 128 nodes

_compiled_nc = None
_last_in_maps = None


def _build(reps=1):
    import concourse.bacc as bacc
    import concourse.mybir as mybir
    import concourse.tile as tile

    f32 = mybir.dt.float32

    nc = bacc.Bacc("TRN2", target_bir_lowering=False, debug=False)

    rowsT = nc.dram_tensor("rowsT", [N, B_LOC], f32, kind="ExternalInput")
    xt = nc.dram_tensor("xt", [D_IN, N], f32, kind="ExternalInput")
    w = nc.dram_tensor("w", [D_IN, D_H], f32, kind="ExternalInput")
    bias = nc.dram_tensor("bias", [128, D_H], f32, kind="ExternalInput")
    outT = nc.dram_tensor("outT", [D_H, B_LOC], f32, kind="ExternalOutput")

    with tile.TileContext(nc) as tc:
        with (
            tc.tile_pool(name="const", bufs=1) as cpool,
            tc.tile_pool(name="xtp", bufs=1) as xtpool,
            tc.tile_pool(name="enc", bufs=KC) as encpool,
            tc.tile_pool(name="rows", bufs=8) as rpool,
            tc.tile_pool(name="res", bufs=2) as opool,
            tc.tile_pool(name="psenc", bufs=4, space="PSUM") as psenc,
            tc.tile_pool(name="psout", bufs=2, space="PSUM") as psout,
        ):
            for _rep in range(reps):
                w_sb = cpool.tile([D_IN, D_H], f32, tag="w")
                nc.sync.dma_start(w_sb[:], w[:])
                bias_sb = cpool.tile([128, D_H], f32, tag="bias")
                nc.sync.dma_start(bias_sb[:], bias[:])

                xt_sb = xtpool.tile([D_IN, N], f32, tag="xt")
                for j in range(8):
                    s = slice(j * (N // 8), (j + 1) * (N // 8))
                    nc.sync.dma_start(xt_sb[:, s], xt[:, s])

                # encoder: enc[n, h] = X @ W + b, chunked over n
                enc_tiles = []
                for k in range(KC):
                    pe = psenc.tile([128, D_H], f32, tag="psenc")
                    nc.tensor.matmul(
                        pe[:],
                        xt_sb[:, k * 128 : (k + 1) * 128],
                        w_sb[:],
                        start=True,
                        stop=True,
                    )
                    et = encpool.tile([128, D_H], f32, tag="enc")
                    nc.vector.tensor_add(et[:], pe[:], bias_sb[:])
                    enc_tiles.append(et)

                # diffusion GEMM: outT[h, b] = sum_k over k-chunks
                out_ps = psout.tile([D_H, B_LOC], f32, tag="psout")
                for k in range(KC):
                    rt = rpool.tile([128, B_LOC], f32, tag="rows")
                    nc.sync.dma_start(rt[:], rowsT[k * 128 : (k + 1) * 128, :])
                    nc.tensor.matmul(
                        out_ps[:],
                        enc_tiles[k][:],
                        rt[:],
                        start=(k == 0),
                        stop=(k == KC - 1),
                    )

                outT_sb = opool.tile([D_H, B_LOC], f32, tag="res")
                nc.vector.tensor_copy(outT_sb[:], out_ps[:])
                nc.sync.dma_start(outT[:], outT_sb[:])

    nc.compile()
    return nc


def prepare_in_maps(X, ppr, W, b, idx):
    from concurrent.futures import ThreadPoolExecutor

    X = np.asarray(X, dtype=np.float32)
    ppr = np.asarray(ppr, dtype=np.float32)
    W = np.asarray(W, dtype=np.float32)
    b = np.asarray(b, dtype=np.float32)
    idx = np.asarray(idx).astype(np.int64)

    xt = np.ascontiguousarray(X.T)
    bias_bc = np.ascontiguousarray(np.broadcast_to(b, (128, D_H)))

    def _rows_for_core(c):
        sel = idx[c * B_LOC : (c + 1) * B_LOC]
        return np.ascontiguousarray(ppr[sel].T)

    with ThreadPoolExecutor(N_CORES) as ex:
        rowsT_per_core = list(ex.map(_rows_for_core, range(N_CORES)))

    return [
        {"rowsT": rowsT_per_core[c], "xt": xt, "w": W, "bias": bias_bc}
        for c in range(N_CORES)
    ]


def kernel(X, ppr, W, b, idx):
    from concourse.bass_utils import run_bass_kernel_spmd

    global _compiled_nc
    if _compiled_nc is None:
        _compiled_nc = _build()
    nc = _compiled_nc

    in_maps = prepare_in_maps(X, ppr, W, b, idx)

    global _last_in_maps
    _last_in_maps = in_maps

    res = run_bass_kernel_spmd(nc, in_maps, list(range(N_CORES))).results
    out = np.concatenate([res[c]["outT"].T for c in range(N_CORES)], axis=0)
    return np.ascontiguousarray(out, dtype=np.float32)
